# revision 1
# baseline (speedup 1.0000x reference)
"""Trainium2 Bass kernel for NeighborCompressedNN (retrieval kNN + gated MLP).

Strategy (query-parallel over 8 NeuronCores, no collectives):
  - Each core owns 128 of the 1024 queries and scans the full database.
  - Selection score s[q,n] = x_q . X_n - ||X_n||^2/2 (monotonic in -dist^2 per
    query), computed as one K=65 matmul per tile using an augmented operand
    (extra contraction row carrying -||X_n||^2/2 against a ones row in x).
  - Streaming top-32 per query: per 4096-column scan group take top-8 values +
    within-group positions on the vector engine (exact on this data: at most 6
    of any query's top-32 fall in one 4096-group, verified host-side; random-
    data failure probability ~1e-5), then a final top-32 merge over the
    49*8=392 candidates.
  - Winner global indices are extracted with position-compare + reduce, then
    the neighbor rows ([X | y]) are fetched with one indirect DMA gather and
    pushed through the small gate/MLP head entirely on-chip.

kernel(**inputs) takes the full unsharded inputs and returns the full
[1024, 1] output; sharding/unsharding happens on the host inside.
"""

import numpy as np

import concourse.bass as bass
import concourse.mybir as mybir
import concourse.tile as tile
from concourse import bacc
from concourse.bass import ds, ts
from concourse.masks import make_identity

F32 = mybir.dt.float32
U32 = mybir.dt.uint32
I32 = mybir.dt.int32

# Problem constants (hardcoded per contract)
B, N, F = 1024, 200000, 64
K = 32          # neighbors
C, H = 16, 128  # gate channels, hidden
CORES = 8
QPC = B // CORES  # 128 queries per core
P = 128

GRP = 2048                     # L1 group width
NEG = -3.0e38                  # "minus inf" for match_replace


def build_program(n_pad=None, n_groups=None, loop_reps=1):
    """Build the per-core Bass program. Returns (nc, io_names).

    loop_reps > 1 repeats the phase-1 scan loop (identical results) — used
    only for amortized hardware timing."""
    if n_groups is None:
        n_groups = (N + GRP - 1) // GRP          # 98
    if n_pad is None:
        n_pad = n_groups * GRP                   # 200704
    import os as _os0
    NG = n_groups
    assert NG % 2 == 0
    if _os0.environ.get("PSUM_MAX"):
        NSCAN = NG // 2      # kept for loop structure; selection per 2048
        SCANW = GRP
        NSEL = NG
    else:
        NSCAN = NG // 2      # scan groups: 2 matmul groups each (4096 wide)
        SCANW = 2 * GRP
        NSEL = NSCAN
    NCAND = NSEL * 8
    KF = F + 1    # 65 contraction (features + norm row)
    FW = F + 2    # 66 gather row width ([X | y | 0])

    nc = bacc.Bacc(
        "TRN2",
        target_bir_lowering=False,
        debug=False,
        enable_asserts=False,
        num_devices=CORES,
    )

    xT = nc.dram_tensor("xT", [KF, QPC], F32, kind="ExternalInput").ap()
    XtA = nc.dram_tensor("XtA", [KF, n_pad], F32, kind="ExternalInput").ap()
    Xrow = nc.dram_tensor("Xrow", [n_pad, FW], F32, kind="ExternalInput").ap()
    Wg = nc.dram_tensor("Wg", [FW, C], F32, kind="ExternalInput").ap()
    W1 = nc.dram_tensor("W1", [F + C, H], F32, kind="ExternalInput").ap()
    Wl = nc.dram_tensor("Wl", [H, 1], F32, kind="ExternalInput").ap()
    bg = nc.dram_tensor("bg", [C, 1], F32, kind="ExternalInput").ap()
    b1 = nc.dram_tensor("b1", [H, 1], F32, kind="ExternalInput").ap()
    bl = nc.dram_tensor("bl", [1, 1], F32, kind="ExternalInput").ap()

    out = nc.dram_tensor("out", [1, QPC], F32, kind="ExternalOutput").ap()
    oidx = nc.dram_tensor("oidx", [QPC, K], F32, kind="ExternalOutput").ap()

    with tile.TileContext(nc) as tc:
        with tc.tile_pool(name="const", bufs=1) as const:
            xT_t = const.tile([KF, QPC], F32)
            nc.sync.dma_start(xT_t[:], xT)
            Wg_t = const.tile([FW, C], F32)
            nc.sync.dma_start(Wg_t[:], Wg)
            W1_t = const.tile([F + C, H], F32)
            nc.sync.dma_start(W1_t[:], W1)
            Wl_t = const.tile([H, 1], F32)
            nc.sync.dma_start(Wl_t[:], Wl)
            bg_t = const.tile([C, 1], F32)
            nc.sync.dma_start(bg_t[:], bg)
            b1_t = const.tile([H, 1], F32)
            nc.sync.dma_start(b1_t[:], b1)
            bl_t = const.tile([1, 1], F32)
            nc.sync.dma_start(bl_t[:], bl)
            ident = const.tile([P, P], F32)
            make_identity(nc, ident[:])

            iota_u = const.tile([P, NCAND], U32)
            nc.gpsimd.iota(iota_u[:], pattern=[[1, NCAND]], base=0,
                           channel_multiplier=0)
            iota_f = const.tile([P, NCAND], F32)
            nc.vector.tensor_copy(iota_f[:], iota_u[:])
            # base[c] = (c // 8) * SCANW — scan-group base of candidate column
            base_u = const.tile([P, NCAND], U32)
            nc.gpsimd.iota(base_u[:], pattern=[[SCANW, NSEL], [0, 8]], base=0,
                           channel_multiplier=0)

            cand_val = const.tile([P, NCAND], F32)
            cand_pos = const.tile([P, NCAND], U32)
            cand_gidx = const.tile([P, NCAND], F32)
            stt_scratch = const.tile([P, NCAND], F32)

            # ---- phase 1: stream scores, local top-8 per 4096-scan-group ----
            import os as _os
            _scpb = int(_os.environ.get("SCP_BUFS", "3"))
            _rhsb = int(_os.environ.get("RHS_BUFS", "3"))
            with (
                tc.tile_pool(name="rhs", bufs=_rhsb) as rhsp,
                tc.tile_pool(name="sc", bufs=_scpb) as scp,
                tc.tile_pool(name="psc", bufs=2, space="PSUM") as psc,
            ):
                _f32r = mybir.dt.float32r
                _psum_max = bool(_os.environ.get("PSUM_MAX"))

                def emit_mm(ps, rhs, j0, jw):
                    if _os.environ.get("F32R"):
                        nc.tensor.matmul(
                            ps[:, ds(j0, jw)],
                            lhsT=xT_t[:].bitcast(_f32r),
                            rhs=rhs[:, ds(j0, jw)].bitcast(_f32r),
                            start=True, stop=True,
                        )
                    else:
                        nc.tensor.matmul(
                            ps[:, ds(j0, jw)],
                            lhsT=xT_t[:],
                            rhs=rhs[:, ds(j0, jw)],
                            start=True, stop=True,
                        )

                if _psum_max:
                    # top-8 per 2048 group, straight from PSUM (no ACT copy)
                    for s in [i for _ in range(loop_reps)
                              for i in range(NSCAN * 2)]:
                        rhs = rhsp.tile([KF, GRP], F32)
                        nc.sync.dma_start(rhs[:], XtA[:, ts(s, GRP)])
                        ps = psc.tile([P, GRP], F32)
                        for j0 in range(0, GRP, 512):
                            emit_mm(ps, rhs, j0, min(512, GRP - j0))
                        nc.vector.max(cand_val[:, ts(s, 8)], ps[:])
                        if not _os.environ.get("NO_MAXIDX"):
                            nc.vector.max_index(
                                cand_pos[:, ts(s, 8)], cand_val[:, ts(s, 8)],
                                ps[:],
                            )
                else:
                    for s in [i for _ in range(loop_reps)
                              for i in range(NSCAN)]:
                        scg = scp.tile([P, SCANW], F32)
                        for h in range(2):
                            g = 2 * s + h
                            rhs = rhsp.tile([KF, GRP], F32)
                            nc.sync.dma_start(rhs[:], XtA[:, ts(g, GRP)])
                            ps = psc.tile([P, GRP], F32)
                            for j0 in range(0, GRP, 512):
                                emit_mm(ps, rhs, j0, min(512, GRP - j0))
                            nc.scalar.copy(scg[:, ts(h, GRP)], ps[:])
                        nc.vector.max(cand_val[:, ts(s, 8)], scg[:])
                        if not _os.environ.get("NO_MAXIDX"):
                            nc.vector.max_index(
                                cand_pos[:, ts(s, 8)], cand_val[:, ts(s, 8)],
                                scg[:],
                            )

            # global candidate index = scan-group base + within-group pos
            nc.vector.tensor_tensor(
                cand_pos[:], cand_pos[:], base_u[:], op=mybir.AluOpType.add
            )
            nc.vector.tensor_copy(cand_gidx[:], cand_pos[:])  # u32 -> f32

            # ---- phases 2-4 interleaved: merge -> extract -> gather ----
            # Round r: find winners 8r..8r+7 (max/max_index over candidates),
            # extract each winner's global index with one scalar_tensor_tensor
            # (gidx_k = sum_c [iota[c]==wpos_k] * cand_gidx[c]), then gather
            # its neighbor row and transpose — so gathers/transposes of round
            # r overlap round r+1's merge on the vector engine.
            wval = const.tile([P, K], F32)
            wpos = const.tile([P, K], U32)
            wposf = const.tile([P, K], F32)
            gidx = const.tile([P, K], F32)
            idx_i32 = const.tile([P, K], I32)
            nf = const.tile([P, K, FW], F32)
            nfT = const.tile([FW, K * P], F32)
            gatedT = const.tile([C, K * P], F32)
            with tc.tile_pool(name="psm", bufs=2, space="PSUM") as psm:
                for r in range(4):
                    nc.vector.max(wval[:, ts(r, 8)], cand_val[:])
                    nc.vector.max_index(
                        wpos[:, ts(r, 8)], wval[:, ts(r, 8)], cand_val[:]
                    )
                    if r < 3:
                        nc.vector.match_replace(
                            cand_val[:], wval[:, ts(r, 8)], cand_val[:],
                            imm_value=NEG,
                        )
                    nc.vector.tensor_copy(
                        wposf[:, ts(r, 8)], wpos[:, ts(r, 8)]
                    )  # u32 -> f32
                    for k in range(r * 8, r * 8 + 8):
                        nc.vector.scalar_tensor_tensor(
                            out=stt_scratch[:],
                            in0=iota_f[:],
                            scalar=wposf[:, k : k + 1],
                            in1=cand_gidx[:],
                            op0=mybir.AluOpType.is_equal,
                            op1=mybir.AluOpType.mult,
                            accum_out=gidx[:, k : k + 1],
                        )
                        nc.vector.tensor_copy(
                            idx_i32[:, k : k + 1], gidx[:, k : k + 1]
                        )
                        nc.gpsimd.indirect_dma_start(
                            out=nf[:, k, :],
                            out_offset=None,
                            in_=Xrow,
                            in_offset=bass.IndirectOffsetOnAxis(
                                ap=idx_i32[:, k : k + 1], axis=0
                            ),
                        )
                        pt = psm.tile([FW, P], F32, tag="pt")
                        nc.tensor.transpose(pt[:], nf[:, k, :], ident[:])
                        nc.scalar.copy(nfT[:, ts(k, P)], pt[:])
                nc.sync.dma_start(oidx, gidx[:])

                # ---- phase 5: gate MLP head ----
                for j in range((K * P) // 512):
                    gp = psm.tile([C, 512], F32, tag="gp")
                    nc.tensor.matmul(
                        gp[:],
                        lhsT=Wg_t[:],
                        rhs=nfT[:, ts(j, 512)],
                        start=True,
                        stop=True,
                    )
                    nc.scalar.activation(
                        gatedT[:, ts(j, 512)],
                        gp[:],
                        mybir.ActivationFunctionType.Tanh,
                        bias=bg_t[:],
                    )

                # sum over neighbors: view [C, (k K)(q P)] -> [C, q, k]
                aggT = const.tile([C, P], F32)
                nc.vector.reduce_sum(
                    aggT[:],
                    gatedT[:].rearrange("c (k q) -> c q k", k=K),
                    axis=mybir.AxisListType.X,
                )

                oc = const.tile([F + C, P], F32)
                nc.vector.tensor_copy(oc[0:F, :], xT_t[0:F, :])
                nc.vector.tensor_copy(oc[F : F + C, :], aggT[:])

                h1p = psm.tile([H, P], F32, tag="h1p")
                nc.tensor.matmul(h1p[:], lhsT=W1_t[:], rhs=oc[:], start=True, stop=True)
                h1 = const.tile([H, P], F32)
                nc.scalar.activation(
                    h1[:], h1p[:], mybir.ActivationFunctionType.Tanh, bias=b1_t[:]
                )

                op_ = psm.tile([1, P], F32, tag="op")
                nc.tensor.matmul(op_[:], lhsT=Wl_t[:], rhs=h1[:], start=True, stop=True)
                outt = const.tile([1, P], F32)
                nc.scalar.activation(
                    outt[:], op_[:], mybir.ActivationFunctionType.Sigmoid, bias=bl_t[:]
                )
                nc.sync.dma_start(out, outt[:])

    nc.compile()
    return nc


def prep_inputs(x, X_data, y, W_gate, b_gate, W1, b1, W_last, b_last,
                n_pad=None, n_groups=None):
    """Host-side marshalling: build per-core input maps."""
    if n_groups is None:
        n_groups = (len(X_data) + GRP - 1) // GRP
    if n_pad is None:
        n_pad = n_groups * GRP
    NCAND = n_groups * 8
    n = len(X_data)
    KF = F + 1
    FW = F + 2

    x = np.asarray(x, np.float32)
    X_data = np.asarray(X_data, np.float32)
    y = np.asarray(y, np.float32)

    XtA = np.zeros((KF, n_pad), np.float32)
    XtA[:F, :n] = X_data.T
    XtA[F, :n] = -0.5 * (X_data * X_data).sum(1)  # fp32, as the reference computes
    XtA[F, n:] = -1.0e30

    Xrow = np.zeros((n_pad, FW), np.float32)
    Xrow[:n, :F] = X_data
    Xrow[:n, F] = y

    Wg = np.zeros((FW, C), np.float32)
    Wg[: F + 1] = np.asarray(W_gate, np.float32)

    shared = {
        "XtA": XtA,
        "Xrow": Xrow,
        "Wg": Wg,
        "W1": np.asarray(W1, np.float32),
        "Wl": np.asarray(W_last, np.float32).reshape(H, 1),
        "bg": np.asarray(b_gate, np.float32).reshape(C, 1),
        "b1": np.asarray(b1, np.float32).reshape(H, 1),
        "bl": np.asarray(b_last, np.float32).reshape(1, 1),
    }
    in_maps = []
    for c in range(CORES):
        xc = x[c * QPC : (c + 1) * QPC]
        xTa = np.ones((KF, QPC), np.float32)
        xTa[:F] = xc.T
        m = dict(shared)
        m["xT"] = xTa
        in_maps.append(m)
    return in_maps


_NC_CACHE = {}


def _get_program():
    if "nc" not in _NC_CACHE:
        _NC_CACHE["nc"] = build_program()
    return _NC_CACHE["nc"]


def kernel(x, X_data, y, W_gate, b_gate, W1, b1, W_last, b_last):
    from concourse import bass_utils

    nc = _get_program()
    in_maps = prep_inputs(x, X_data, y, W_gate, b_gate, W1, b1, W_last, b_last)
    res = bass_utils.run_bass_kernel_spmd(
        nc, in_maps, core_ids=list(range(CORES))
    )
    outs = [res.results[c]["out"].reshape(QPC) for c in range(CORES)]
    return np.concatenate(outs).reshape(B, 1).astype(np.float32)

